# revision 2
# baseline (speedup 1.0000x reference)
"""KNN graph kernel (DenseDilatedKnnGraph) for Trainium2, 8 NeuronCores.

Problem: x [2, 192, 8192, 1] fp32 -> edge_index [2, 2, 8192, 9] int32.
reference: L2-normalize x along C, pairwise sq-dists over N, top-9 (k=9,
dilation=1) nearest neighbors (indices), stacked with center indices.

Math: for normalized points, ranking by -dist == ranking by cosine
G = Xn^T Xn. Device computes, per query row, the comb-max vector
V[p] = max_m G[q, p + 1024*m] (1024 combs of 8 columns) and ships V to
the host. Host takes the top-16 combs per row (always contains every
comb holding a true top-9 column, since a comb's max is >= the 9th
value and device/host value skew is ~2e-4 fp16 rounding), rescores the
128 candidate columns exactly in fp64, and emits jax-top_k order.

Device engine split per 128-query row tile (~5.1-5.5 us each):
  - PE: Gram in 4 PSUM quarters [128, 2048]. K=192 packed as one
    K=128 matmul plus a K=64 matmul pair row-tiled at array positions
    (0,0)/(64,0) running concurrently (weights+rhs duplicated across
    partition halves), accumulating into the same PSUM banks.
    ~12.3k PE cycles/tile instead of 16.4k for K=256 zero-padding.
  - ACT: evacuates quarters 0,1 and 3/4 of quarter 2 to fp16.
  - DVE: first fold level fused with PSUM reads for the rest
    (tensor_max with one PSUM operand), then folds to V [128, 1024].
  - DMA: V -> HBM (vout), 256 KB per tile.
"""

import numpy as np

B = 2
C = 192
N = 8192
NCORES = 8
RBLK = N // 4  # 2048 query rows per core
NT = RBLK // 128  # 16 row tiles per core
NV = 1024  # V width; comb(p) = {p + 1024*m : m = 0..7}
TCOMB = 16  # combs the host rescores per row

_cache = {}


def _build_nc():
    import concourse.bacc as bacc
    import concourse.mybir as mybir
    from concourse.bass import ts
    from concourse.tile import TileContext

    f32 = mybir.dt.float32
    f16 = mybir.dt.float16

    nc = bacc.Bacc("TRN2")

    # full (per-batch) normalized points + this core's query block
    xin = nc.dram_tensor("xin", [C, N], f16, kind="ExternalInput")
    wq = nc.dram_tensor("wq", [C, RBLK], f16, kind="ExternalInput")
    vout = nc.dram_tensor("vout", [RBLK, NV], f16, kind="ExternalOutput")

    DCH = 1024  # input DMA chunk

    with TileContext(nc) as tc:
        with (
            tc.tile_pool(name="xpool", bufs=1) as xpool,
            tc.tile_pool(name="gpool", bufs=2) as gpool,
            tc.tile_pool(name="fpool", bufs=2) as fpool,
            tc.tile_pool(name="vpool", bufs=3) as vpool,
            tc.tile_pool(name="gpsum", bufs=2, space="PSUM") as gpsum,
        ):
            # query-block weights: channels 0-127, and channels 128-191
            # duplicated across both partition halves for the row-tiled
            # K=64 matmul pair.
            hAq = xpool.tile([128, RBLK], f16)
            hBq = xpool.tile([128, RBLK], f16)
            nc.sync.dma_start(hAq, wq[0:128, :])
            nc.sync.dma_start(hBq[0:64, :], wq[128:192, :])
            nc.sync.dma_start(hBq[64:128, :], wq[128:192, :])

            # moving operand: all N columns, same duplication for hB.
            hA = xpool.tile([128, N], f16)
            hB = xpool.tile([128, N], f16)
            for sc in range(4):  # first chunk split for an early start
                ssl = ts(sc, 256)
                nc.sync.dma_start(hA[:, ssl], xin[0:128, ssl])
                nc.sync.dma_start(hB[0:64, ssl], xin[128:192, ssl])
                nc.sync.dma_start(hB[64:128, ssl], xin[128:192, ssl])
            for dc in range(1, N // DCH):
                dsl = ts(dc, DCH)
                nc.sync.dma_start(hA[:, dsl], xin[0:128, dsl])
                nc.sync.dma_start(hB[0:64, dsl], xin[128:192, dsl])
                nc.sync.dma_start(hB[64:128, dsl], xin[128:192, dsl])

            for t in range(NT):
                tsl = ts(t, 128)
                g0 = gpool.tile([128, 2048], f16, tag="g0")
                g1 = gpool.tile([128, 2048], f16, tag="g1")
                g2 = gpool.tile([128, 1536], f16, tag="g2")
                F1a = fpool.tile([128, 2048], f16, tag="F1a")
                F1b = fpool.tile([128, 2048], f16, tag="F1b")
                for i in range(4):
                    ps = gpsum.tile([128, 2048], f32, tag="ps")
                    # K=128 pass (channels 0-127), one matmul per PSUM bank
                    for hh in range(4):
                        csl = ts(4 * i + hh, 512)
                        osl = slice(hh * 512, (hh + 1) * 512)
                        nc.tensor.matmul(
                            ps[:, osl], hAq[:, tsl], hA[:, csl],
                            start=True, stop=False,
                        )
                    # K=64 pass (channels 128-191) as row-tiled pairs:
                    # consecutive matmuls on disjoint row groups overlap.
                    for hh in range(4):
                        csl = ts(4 * i + hh, 512)
                        osl = slice(hh * 512, (hh + 1) * 512)
                        pr = slice(0, 64) if hh % 2 == 0 else slice(64, 128)
                        nc.tensor.matmul(
                            ps[:, osl], hBq[pr, tsl], hB[pr, csl],
                            start=False, stop=True,
                        )
                    if i == 0:
                        nc.scalar.copy(g0, ps)
                    elif i == 1:
                        nc.scalar.copy(g1, ps)
                    elif i == 2:
                        nc.scalar.copy(g2, ps[:, 0:1536])
                        nc.vector.tensor_max(
                            F1a[:, 0:1536], g0[:, 0:1536], g2
                        )
                        nc.vector.tensor_max(
                            F1a[:, 1536:2048], ps[:, 1536:2048],
                            g0[:, 1536:2048],
                        )
                    else:
                        nc.vector.tensor_max(F1b, ps, g1)
                F2 = vpool.tile([128, 2048], f16, tag="F2")
                nc.vector.tensor_max(F2, F1a, F1b)
                V = vpool.tile([128, NV], f16, tag="V")
                nc.vector.tensor_max(V, F2[:, 0:NV], F2[:, NV : 2 * NV])
                nc.sync.dma_start(vout[tsl, :], V)

    nc.compile()
    return nc


def _get_nc():
    if "nc" not in _cache:
        _cache["nc"] = _build_nc()
    return _cache["nc"]


def shard_inputs(x):
    """x: [B, C, N, 1] -> 8 per-core inputs: normalized fp16 points
    (full batch) + the core's own 2048-column query block."""
    xs = np.ascontiguousarray(np.asarray(x, dtype=np.float32).reshape(B, C, N))
    rns = 1.0 / np.sqrt((xs * xs).sum(axis=1, keepdims=True))  # [B, 1, N]
    h16 = (xs * rns).astype(np.float16)
    in_maps = []
    for c in range(NCORES):
        b, r = divmod(c, 4)
        s = r * RBLK
        in_maps.append(
            {"xin": h16[b], "wq": np.ascontiguousarray(h16[b][:, s : s + RBLK])}
        )
    return in_maps


def assemble(results, x):
    """results: 8 dicts with 'vout' [RBLK, NV] f16 comb-max vectors.

    comb(p) = {p + 1024*m : m = 0..7}. Take top-TCOMB combs per row,
    rescore all TCOMB*8 candidate columns with exact fp64 dots of the
    normalized points, and take the true top-8 by (-value, index).
    """
    xs = np.asarray(x, dtype=np.float32).reshape(B, C, N)
    n64 = np.sqrt((xs.astype(np.float64) ** 2).sum(axis=1, keepdims=True))
    xn = np.ascontiguousarray((xs / n64).transpose(0, 2, 1))  # [B, N, C] f64

    nn = np.empty((B, N, 9), np.int32)
    m_off = (np.arange(8, dtype=np.int64) * NV)[None, None, :]
    for c in range(NCORES):
        b, r = divmod(c, 4)
        s = r * RBLK
        V = results[c]["vout"]  # [RBLK, NV] f16
        combs = np.argpartition(-V, TCOMB, axis=1)[:, :TCOMB].astype(np.int64)
        cand = (combs[:, :, None] + m_off).reshape(RBLK, TCOMB * 8)
        rows = np.arange(s, s + RBLK, dtype=np.int64)
        xnb = xn[b]
        top8 = np.empty((RBLK, 8), np.int64)
        CH = 512
        for r0 in range(0, RBLK, CH):
            cc = cand[r0 : r0 + CH]
            rr = rows[r0 : r0 + CH]
            vals = np.einsum("rkc,rc->rk", xnb[cc], xnb[rr], optimize=True)
            vals[cc == rr[:, None]] = -np.inf
            order = np.lexsort((cc, -vals), axis=-1)[:, :8]
            top8[r0 : r0 + CH] = np.take_along_axis(cc, order, axis=1)
        nn[b, s : s + RBLK, 1:9] = top8
        nn[b, s : s + RBLK, 0] = rows
    center = np.broadcast_to(np.arange(N, dtype=np.int32)[None, :, None], (B, N, 9))
    return np.ascontiguousarray(np.stack([nn, center], axis=0).astype(np.int32))


def kernel(x, _trace=False, **trace_kwargs):
    from concourse.bass_utils import run_bass_kernel_spmd

    nc = _get_nc()
    in_maps = shard_inputs(x)
    res = run_bass_kernel_spmd(
        nc, in_maps, core_ids=list(range(NCORES)), trace=_trace, **trace_kwargs
    )
    _cache["last_results"] = res
    return assemble(res.results, x)


# revision 6
# speedup vs baseline: 1.2320x; 1.2320x over previous
"""KNN graph kernel (DenseDilatedKnnGraph) for Trainium2, 8 NeuronCores.

Problem: x [2, 192, 8192, 1] fp32 -> edge_index [2, 2, 8192, 9] int32.
reference: L2-normalize x along C, pairwise sq-dists over N, top-9 (k=9,
dilation=1) nearest neighbors (indices), stacked with center indices.

Math: for normalized points, ranking by -dist == ranking by cosine
G = Xn^T Xn. Device computes, per query row, the comb-max vector
F2[p] = max_m G[q, p + 2048*m] (2048 combs of 4 columns) and ships F2
to the host. Host takes the top-24 combs per row (always contains every
comb holding a true top-9 column: a comb's max is >= the 9th value, and
device/host value skew is only the ~2e-4 fp16 rounding), rescores the
96 candidate columns exactly in fp64, and emits jax-top_k order.

Device schedule per 128-query row tile (~5.1-5.7 us per engine):
  - PE: Gram in 8 PSUM eighths [128, 1024] (bufs=4 -> depth-4
    pipeline). K=192 packed as a K=64 matmul pair row-tiled at array
    positions (0,0)/(64,0) running concurrently (weights+rhs duplicated
    across partition halves) plus a K=128 matmul, accumulating into the
    same PSUM banks. ~12.3k PE cycles/tile vs 16.4k for zero-padding.
  - ACT: evacuates eighths 0-3 and 7 to fp16 (5 ACTIVATEs).
  - DVE: eighths 4,5,6 fold directly from PSUM (tensor_max with one
    PSUM operand = fused evacuate+fold), 7 folds fp16x2, then two
    fp16 2x folds produce F2 [128, 2048].
  - DMA: F2 -> HBM (vout), 512 KB per tile.
"""

import numpy as np

B = 2
C = 192
N = 8192
NCORES = 8
RBLK = N // 4  # 2048 query rows per core
NT = RBLK // 128  # 16 row tiles per core
NV = 2048  # F2 width; comb(p) = {p + 2048*m : m = 0..3}
TCOMB = 24  # combs the host rescores per row

_cache = {}


def _build_nc():
    import concourse.bacc as bacc
    import concourse.mybir as mybir
    from concourse.bass import ts
    from concourse.tile import TileContext

    f32 = mybir.dt.float32
    f16 = mybir.dt.float16

    nc = bacc.Bacc("TRN2")

    # full (per-batch) normalized points + this core's query block
    xin = nc.dram_tensor("xin", [C, N], f16, kind="ExternalInput")
    wq = nc.dram_tensor("wq", [C, RBLK], f16, kind="ExternalInput")
    vout = nc.dram_tensor("vout", [RBLK, NV], f16, kind="ExternalOutput")

    DCH = 1024  # input DMA chunk

    with TileContext(nc) as tc:
        with (
            tc.tile_pool(name="xpool", bufs=1) as xpool,
            tc.tile_pool(name="gpool", bufs=2) as gpool,
            tc.tile_pool(name="fpool", bufs=2) as fpool,
            tc.tile_pool(name="vpool", bufs=3) as vpool,
            tc.tile_pool(name="gpsum", bufs=4, space="PSUM") as gpsum,
        ):
            # query-block weights: channels 0-127, and channels 128-191
            # duplicated across both partition halves for the row-tiled
            # K=64 matmul pair.
            hAq = xpool.tile([128, RBLK], f16)
            hBq = xpool.tile([128, RBLK], f16)
            nc.sync.dma_start(hAq, wq[0:128, :])
            nc.sync.dma_start(hBq[0:64, :], wq[128:192, :])
            nc.sync.dma_start(hBq[64:128, :], wq[128:192, :])

            # moving operand: all N columns, same duplication for hB.
            hA = xpool.tile([128, N], f16)
            hB = xpool.tile([128, N], f16)
            for sc in range(4):  # first chunk split for an early start
                ssl = ts(sc, 256)
                nc.sync.dma_start(hA[:, ssl], xin[0:128, ssl])
                nc.sync.dma_start(hB[0:64, ssl], xin[128:192, ssl])
                nc.sync.dma_start(hB[64:128, ssl], xin[128:192, ssl])
            for dc in range(1, N // DCH):
                dsl = ts(dc, DCH)
                nc.sync.dma_start(hA[:, dsl], xin[0:128, dsl])
                nc.sync.dma_start(hB[0:64, dsl], xin[128:192, dsl])
                nc.sync.dma_start(hB[64:128, dsl], xin[128:192, dsl])

            for t in range(NT):
                tsl = ts(t, 128)
                a = {}
                for j in (0, 1, 2, 3, 7):
                    a[j] = gpool.tile(
                        [128, 1024], f16, tag=f"a{j}", name=f"a{j}"
                    )
                P = {}
                for j in range(4):
                    P[j] = fpool.tile(
                        [128, 1024], f16, tag=f"P{j}", name=f"P{j}"
                    )
                Fv = vpool.tile([128, NV], f16, tag="Fv")

                pst = {}
                # 4 eighth-pairs of matmuls; K=64 pair first (start),
                # K=128 second (stop).  Within the K=64 phase the A/B
                # row-group alternation makes consecutive matmuls
                # overlap in the PE array.
                for pair in range(4):
                    j0, j1 = 2 * pair, 2 * pair + 1
                    psA = gpsum.tile([128, 1024], f32, tag="ps", name="psA")
                    psB = gpsum.tile([128, 1024], f32, tag="ps", name="psB")
                    pst[j0], pst[j1] = psA, psB
                    cs = [ts(4 * pair + hh, 512) for hh in range(4)]
                    for k, (ps, osl, csl) in enumerate(
                        (
                            (psA, slice(0, 512), cs[0]),
                            (psA, slice(512, 1024), cs[1]),
                            (psB, slice(0, 512), cs[2]),
                            (psB, slice(512, 1024), cs[3]),
                        )
                    ):
                        pr = slice(0, 64) if k % 2 == 0 else slice(64, 128)
                        nc.tensor.matmul(
                            ps[:, osl], hBq[pr, tsl], hB[pr, csl],
                            start=True, stop=False,
                        )
                    for ps, osl, csl in (
                        (psA, slice(0, 512), cs[0]),
                        (psA, slice(512, 1024), cs[1]),
                        (psB, slice(0, 512), cs[2]),
                        (psB, slice(512, 1024), cs[3]),
                    ):
                        nc.tensor.matmul(
                            ps[:, osl], hAq[:, tsl], hA[:, csl],
                            start=False, stop=True,
                        )
                    # consumers, issued as soon as their eighth is done
                    if pair == 0:
                        nc.scalar.copy(a[0], psA)
                        nc.scalar.copy(a[1], psB)
                    elif pair == 1:
                        nc.scalar.copy(a[2], psA)
                        nc.scalar.copy(a[3], psB)
                    elif pair == 2:
                        nc.vector.tensor_max(P[0], psA, a[0])
                        nc.vector.tensor_max(P[1], psB, a[1])
                    else:
                        nc.vector.tensor_max(P[2], psA, a[2])
                        nc.scalar.copy(a[7], psB)
                        nc.vector.tensor_max(P[3], a[7], a[3])
                nc.vector.tensor_max(Fv[:, 0:1024], P[0], P[2])
                nc.vector.tensor_max(Fv[:, 1024:2048], P[1], P[3])
                nc.sync.dma_start(vout[tsl, :], Fv)

    nc.compile()
    return nc


def _get_nc():
    if "nc" not in _cache:
        _cache["nc"] = _build_nc()
    return _cache["nc"]


def shard_inputs(x):
    """x: [B, C, N, 1] -> 8 per-core inputs: normalized fp16 points
    (full batch) + the core's own 2048-column query block."""
    xs = np.ascontiguousarray(np.asarray(x, dtype=np.float32).reshape(B, C, N))
    rns = 1.0 / np.sqrt((xs * xs).sum(axis=1, keepdims=True))  # [B, 1, N]
    h16 = (xs * rns).astype(np.float16)
    in_maps = []
    for c in range(NCORES):
        b, r = divmod(c, 4)
        s = r * RBLK
        in_maps.append(
            {"xin": h16[b], "wq": np.ascontiguousarray(h16[b][:, s : s + RBLK])}
        )
    return in_maps


def assemble(results, x):
    """results: 8 dicts with 'vout' [RBLK, NV] f16 comb-max vectors.

    comb(p) = {p + 2048*m : m = 0..3}. Take top-TCOMB combs per row,
    rescore all TCOMB*4 candidate columns with exact fp64 dots of the
    normalized points, and take the true top-8 by (-value, index).
    """
    xs = np.asarray(x, dtype=np.float32).reshape(B, C, N)
    n64 = np.sqrt((xs.astype(np.float64) ** 2).sum(axis=1, keepdims=True))
    xn = np.ascontiguousarray((xs / n64).transpose(0, 2, 1))  # [B, N, C] f64

    nn = np.empty((B, N, 9), np.int32)
    m_off = (np.arange(4, dtype=np.int64) * NV)[None, None, :]
    for c in range(NCORES):
        b, r = divmod(c, 4)
        s = r * RBLK
        V = results[c]["vout"]  # [RBLK, NV] f16
        combs = np.argpartition(-V, TCOMB, axis=1)[:, :TCOMB].astype(np.int64)
        cand = (combs[:, :, None] + m_off).reshape(RBLK, TCOMB * 4)
        rows = np.arange(s, s + RBLK, dtype=np.int64)
        xnb = xn[b]
        top8 = np.empty((RBLK, 8), np.int64)
        CH = 512
        for r0 in range(0, RBLK, CH):
            cc = cand[r0 : r0 + CH]
            rr = rows[r0 : r0 + CH]
            vals = np.einsum("rkc,rc->rk", xnb[cc], xnb[rr], optimize=True)
            vals[cc == rr[:, None]] = -np.inf
            order = np.lexsort((cc, -vals), axis=-1)[:, :8]
            top8[r0 : r0 + CH] = np.take_along_axis(cc, order, axis=1)
        nn[b, s : s + RBLK, 1:9] = top8
        nn[b, s : s + RBLK, 0] = rows
    center = np.broadcast_to(np.arange(N, dtype=np.int32)[None, :, None], (B, N, 9))
    return np.ascontiguousarray(np.stack([nn, center], axis=0).astype(np.int32))


def kernel(x, _trace=False, **trace_kwargs):
    from concourse.bass_utils import run_bass_kernel_spmd

    nc = _get_nc()
    in_maps = shard_inputs(x)
    res = run_bass_kernel_spmd(
        nc, in_maps, core_ids=list(range(NCORES)), trace=_trace, **trace_kwargs
    )
    _cache["last_results"] = res
    return assemble(res.results, x)


# revision 11
# speedup vs baseline: 1.2393x; 1.0059x over previous
"""KNN graph kernel (DenseDilatedKnnGraph) for Trainium2, 8 NeuronCores.

Problem: x [2, 192, 8192, 1] fp32 -> edge_index [2, 2, 8192, 9] int32.
reference: L2-normalize x along C, pairwise sq-dists over N, top-9 (k=9,
dilation=1) nearest neighbors (indices), stacked with center indices.

Math: for normalized points, ranking by -dist == ranking by cosine
G = Xn^T Xn. Device computes, per query row, the comb-max vector
F2[p] = max_m G[q, p + 2048*m] (2048 combs of 4 columns) and ships F2
to the host. Host takes the top-24 combs per row (always contains every
comb holding a true top-9 column: a comb's max is >= the 9th value, and
device/host value skew is only the ~2e-4 fp16 rounding), rescores the
96 candidate columns exactly in fp64, and emits jax-top_k order.

Device schedule per 128-query row tile (~5.1-5.7 us per engine):
  - PE: Gram in 8 PSUM eighths [128, 1024] (bufs=4 -> depth-4
    pipeline). K=192 packed as a K=64 matmul pair row-tiled at array
    positions (0,0)/(64,0) running concurrently (weights+rhs duplicated
    across partition halves) plus a K=128 matmul, accumulating into the
    same PSUM banks. ~12.3k PE cycles/tile vs 16.4k for zero-padding.
  - ACT: evacuates eighths 0-3 and 7 to fp16 (5 ACTIVATEs).
  - DVE: eighths 4,5,6 fold directly from PSUM (tensor_max with one
    PSUM operand = fused evacuate+fold), 7 folds fp16x2, then two
    fp16 2x folds produce F2 [128, 2048].
  - DMA: F2 -> HBM (vout), 512 KB per tile.
"""

import numpy as np

B = 2
C = 192
N = 8192
NCORES = 8
RBLK = N // 4  # 2048 query rows per core
NT = RBLK // 128  # 16 row tiles per core
NV = 2048  # F2 width; comb(p) = {p + 2048*m : m = 0..3}
TCOMB = 24  # combs the host rescores per row

_cache = {}


def _build_nc():
    import concourse.bacc as bacc
    import concourse.mybir as mybir
    from concourse.bass import ts
    from concourse.tile import TileContext

    f32 = mybir.dt.float32
    f16 = mybir.dt.float16

    nc = bacc.Bacc("TRN2")

    # full (per-batch) normalized points + this core's query block
    xin = nc.dram_tensor("xin", [C, N], f16, kind="ExternalInput")
    wq = nc.dram_tensor("wq", [C, RBLK], f16, kind="ExternalInput")
    vout = nc.dram_tensor("vout", [RBLK, NV], f16, kind="ExternalOutput")

    DCH = 1024  # input DMA chunk

    with TileContext(nc) as tc:
        with (
            tc.tile_pool(name="xpool", bufs=1) as xpool,
            tc.tile_pool(name="gpool", bufs=3) as gpool,
            tc.tile_pool(name="fpool", bufs=3) as fpool,
            tc.tile_pool(name="vpool", bufs=3) as vpool,
            tc.tile_pool(name="gpsum", bufs=4, space="PSUM") as gpsum,
        ):
            # query-block weights: channels 0-127, and channels 128-191
            # duplicated across both partition halves for the row-tiled
            # K=64 matmul pair.
            hAq = xpool.tile([128, RBLK], f16)
            hBq = xpool.tile([128, RBLK], f16)
            nc.sync.dma_start(hAq, wq[0:128, :])
            nc.sync.dma_start(hBq[0:64, :], wq[128:192, :])
            nc.sync.dma_start(hBq[64:128, :], wq[128:192, :])

            # moving operand: all N columns, same duplication for hB.
            hA = xpool.tile([128, N], f16)
            hB = xpool.tile([128, N], f16)
            for sc in range(4):  # first chunk split for an early start
                ssl = ts(sc, 256)
                nc.sync.dma_start(hA[:, ssl], xin[0:128, ssl])
                nc.sync.dma_start(hB[0:64, ssl], xin[128:192, ssl])
                nc.sync.dma_start(hB[64:128, ssl], xin[128:192, ssl])
            for dc in range(1, N // DCH):
                dsl = ts(dc, DCH)
                nc.sync.dma_start(hA[:, dsl], xin[0:128, dsl])
                nc.sync.dma_start(hB[0:64, dsl], xin[128:192, dsl])
                nc.sync.dma_start(hB[64:128, dsl], xin[128:192, dsl])

            for t in range(NT):
                tsl = ts(t, 128)
                a = {}
                for j in (0, 1, 2, 3, 7):
                    a[j] = gpool.tile(
                        [128, 1024], f16, tag=f"a{j}", name=f"a{j}"
                    )
                P = {}
                for j in range(4):
                    P[j] = fpool.tile(
                        [128, 1024], f16, tag=f"P{j}", name=f"P{j}"
                    )
                Fv = vpool.tile([128, NV], f16, tag="Fv")

                # Per eighth-pair: the K=64 channel remainder runs as
                # row-tiled concurrent matmul pairs (rows 0-63 / 64-127,
                # different PSUM banks -> no accumulate collision), then
                # full-array K=128 matmuls finish each bank (2 writers
                # per bank).
                lo, hi = slice(0, 64), slice(64, 128)
                for pair in range(4):
                    psA = gpsum.tile([128, 1024], f32, tag="ps", name="psA")
                    psB = gpsum.tile([128, 1024], f32, tag="ps", name="psB")
                    cs = [ts(4 * pair + hh, 512) for hh in range(4)]
                    locs = (
                        (psA, slice(0, 512), cs[0]),
                        (psA, slice(512, 1024), cs[1]),
                        (psB, slice(0, 512), cs[2]),
                        (psB, slice(512, 1024), cs[3]),
                    )
                    for k, (ps, osl, csl) in enumerate(locs):
                        pr = lo if k % 2 == 0 else hi
                        nc.tensor.matmul(
                            ps[:, osl], hBq[pr, tsl], hB[pr, csl],
                            start=True, stop=False,
                        )
                    for ps, osl, csl in locs:
                        nc.tensor.matmul(
                            ps[:, osl], hAq[:, tsl], hA[:, csl],
                            start=False, stop=True,
                        )
                    # consumers, issued as soon as their eighth is done
                    if pair == 0:
                        nc.scalar.copy(a[0], psA)
                        nc.scalar.copy(a[1], psB)
                    elif pair == 1:
                        nc.scalar.copy(a[2], psA)
                        nc.scalar.copy(a[3], psB)
                    elif pair == 2:
                        nc.vector.tensor_max(P[0], psA, a[0])
                        nc.vector.tensor_max(P[1], psB, a[1])
                    else:
                        nc.vector.tensor_max(P[2], psA, a[2])
                        nc.scalar.copy(a[7], psB)
                        nc.vector.tensor_max(P[3], a[7], a[3])
                nc.vector.tensor_max(Fv[:, 0:1024], P[0], P[2])
                nc.vector.tensor_max(Fv[:, 1024:2048], P[1], P[3])
                nc.sync.dma_start(vout[tsl, :], Fv)

    nc.compile()
    return nc


def _get_nc():
    if "nc" not in _cache:
        _cache["nc"] = _build_nc()
    return _cache["nc"]


def shard_inputs(x):
    """x: [B, C, N, 1] -> 8 per-core inputs: normalized fp16 points
    (full batch) + the core's own 2048-column query block."""
    xs = np.ascontiguousarray(np.asarray(x, dtype=np.float32).reshape(B, C, N))
    rns = 1.0 / np.sqrt((xs * xs).sum(axis=1, keepdims=True))  # [B, 1, N]
    h16 = (xs * rns).astype(np.float16)
    in_maps = []
    for c in range(NCORES):
        b, r = divmod(c, 4)
        s = r * RBLK
        in_maps.append(
            {"xin": h16[b], "wq": np.ascontiguousarray(h16[b][:, s : s + RBLK])}
        )
    return in_maps


def assemble(results, x):
    """results: 8 dicts with 'vout' [RBLK, NV] f16 comb-max vectors.

    comb(p) = {p + 2048*m : m = 0..3}. Take top-TCOMB combs per row,
    rescore all TCOMB*4 candidate columns with exact fp64 dots of the
    normalized points, and take the true top-8 by (-value, index).
    """
    xs = np.asarray(x, dtype=np.float32).reshape(B, C, N)
    n64 = np.sqrt((xs.astype(np.float64) ** 2).sum(axis=1, keepdims=True))
    xn = np.ascontiguousarray((xs / n64).transpose(0, 2, 1))  # [B, N, C] f64

    nn = np.empty((B, N, 9), np.int32)
    m_off = (np.arange(4, dtype=np.int64) * NV)[None, None, :]
    for c in range(NCORES):
        b, r = divmod(c, 4)
        s = r * RBLK
        V = results[c]["vout"]  # [RBLK, NV] f16
        combs = np.argpartition(-V, TCOMB, axis=1)[:, :TCOMB].astype(np.int64)
        cand = (combs[:, :, None] + m_off).reshape(RBLK, TCOMB * 4)
        rows = np.arange(s, s + RBLK, dtype=np.int64)
        xnb = xn[b]
        top8 = np.empty((RBLK, 8), np.int64)
        CH = 512
        for r0 in range(0, RBLK, CH):
            cc = cand[r0 : r0 + CH]
            rr = rows[r0 : r0 + CH]
            vals = np.einsum("rkc,rc->rk", xnb[cc], xnb[rr], optimize=True)
            vals[cc == rr[:, None]] = -np.inf
            order = np.lexsort((cc, -vals), axis=-1)[:, :8]
            top8[r0 : r0 + CH] = np.take_along_axis(cc, order, axis=1)
        nn[b, s : s + RBLK, 1:9] = top8
        nn[b, s : s + RBLK, 0] = rows
    center = np.broadcast_to(np.arange(N, dtype=np.int32)[None, :, None], (B, N, 9))
    return np.ascontiguousarray(np.stack([nn, center], axis=0).astype(np.int32))


def kernel(x, _trace=False, **trace_kwargs):
    from concourse.bass_utils import run_bass_kernel_spmd

    nc = _get_nc()
    in_maps = shard_inputs(x)
    res = run_bass_kernel_spmd(
        nc, in_maps, core_ids=list(range(NCORES)), trace=_trace, **trace_kwargs
    )
    _cache["last_results"] = res
    return assemble(res.results, x)
